# revision 6
# baseline (speedup 1.0000x reference)
"""Trainium2 Bass kernel: cubic B-spline upsampling x2 of a (2,3,96,96,96) volume.

Math: the reference op (recursive IIR prefilter along each spatial axis, then
an 8-tap stride-2 transposed conv along each axis) is linear and separable.
The whole per-axis operator is a dense 192x96 matrix M (built exactly on the
host in float64).  out = M (x) M (x) M applied along z, y, x.

Device strategy (8 NeuronCores, SPMD, no collectives): the 6 (b,c) volumes
x 192 z'-rows = 1152 output rows are split so every core gets 144 rows from
exactly TWO volumes, with a uniform program:
  pass 0: a 48-row z'-slab of volume 4 or 5   (slab  c%4,  vol 4+(c>=4))
  pass 1: a 96-row z'-half of volume 0..3     (half  c//4, vol c%4)
Each pass: load the volume once (96x96x96 bf16, unpadded), then three
data-stationary matmul stages (stationary = data tile, moving = spline
matrix), so no transposes and no padding anywhere (K=96 contractions):
  A: per x (96 mms): lhsT = vol[z, x-slice, y] (96x96) rhs=MzT(96,w) -> (y, z')
  B: per z' (w mms): lhsT = L1[y, z', x] (96x96)       rhs=MT (96,192) -> (x, y')
  C: per 128-chunk of (z'y') (1.5w mms): lhsT = L2f[x, chunk] rhs=MT -> (chunk, x')
PSUM->SBUF evacuations are 768-elem copies from 2-bank PSUM tiles, spread
ACT:DVE = 5:4 (their throughput ratio). Output DRAM layout is
partition-major [128, 216, 192] so every output DMA descriptor is a >=1.5KB
contiguous run (the 512B threshold below which DMA runs at half speed);
stage tiles of 16 chunks are DMA'd in 2-4 partition-splits, issued
round-robin from the sync/vector/scalar sequencers. Stage C of pass 0 is
partially held back to overlap pass 1's stage A; the last pass ends with
finer stages so the drain is short. Compute in bf16 (PSUM fp32); output
written bf16 and upcast on host (rel err ~5.3e-3 vs the reference).
"""

import math
import os
import sys

import numpy as np

for _p in ("/opt/trn_rl_repo",):
    if _p not in sys.path and os.path.isdir(_p):
        sys.path.insert(0, _p)

import ml_dtypes  # noqa: E402

BF16 = ml_dtypes.bfloat16

POLE = math.sqrt(3.0) - 2.0
GAIN = (1.0 - POLE) * (1.0 - 1.0 / POLE)  # 6.0
N = 96
F = 2
NOUT = N * F  # 192
NCORES = 8

PASS_W = (48, 96)  # z'-rows per pass
PASS_CHUNKS = (72, 144)  # w*192/128
# output stage sizes (chunks per staged DMA); finer at the very end
STAGE_PLAN = ((16, 16, 16, 16, 8), (16, 16, 16, 16, 16, 16, 16, 16, 8, 4, 4))
HOLD0 = 6  # 4-chunk C tiles of pass 0 held back into pass 1's stage A


def _cubic(t):
    a = np.abs(t)
    out = (2.0 / 3.0 + (0.5 * a - 1.0) * a**2) * (a < 1)
    out = out + (-((a - 2.0) ** 3) / 6.0) * ((a >= 1) & (a < 2))
    return out


def _prefilter_mat(n):
    """96x96 matrix of the causal+anticausal cubic-spline prefilter (float64)."""
    p = POLE
    xm = np.eye(n, dtype=np.float64) * GAIN
    i = np.arange(n)
    pows = p**i + p ** (2 * n - 1 - i)
    c = np.zeros((n, n), dtype=np.float64)
    c[0] = (pows @ xm) * (p / (1.0 - p ** (2 * n))) + xm[0]
    for k in range(1, n):
        c[k] = xm[k] + p * c[k - 1]
    out = np.zeros((n, n), dtype=np.float64)
    out[n - 1] = c[n - 1] * (p / (p - 1.0))
    for k in range(n - 2, -1, -1):
        out[k] = p * (out[k + 1] - c[k])
    return out


def _upsample_mat(n, f=F):
    """2n x n matrix of the edge-padded stride-2 transposed conv (float64)."""
    k = 4 * f  # f even -> is_odd == 0
    start = 1.0 / (2 * f) - 2.0
    pts = np.arange(k, dtype=np.float64) * (1.0 / f) + start
    ker = _cubic(pts)
    npad = n + 4
    U = np.zeros((f * n, npad), dtype=np.float64)
    for o in range(f * n):
        for i in range(npad):
            s = o + (k - 1) - f * i
            if 0 <= s < k:
                U[o, i] += ker[s]
    Uc = np.zeros((f * n, n), dtype=np.float64)
    for i in range(npad):
        j = min(max(i - 2, 0), n - 1)
        Uc[:, j] += U[:, i]
    return Uc


def build_M():
    """Exact 192x96 per-axis operator (float64)."""
    return _upsample_mat(N) @ _prefilter_mat(N)


_NC_CACHE = {}


def _strip_redundant_self_waits(nc):
    """Drop sem waits that are trivially satisfied by same-engine program order.

    Tile's per-proc wait emission is not transitively minimal: a PE matmul can
    end up waiting on the PE's own semaphore (already guaranteed by in-order
    engine execution) in addition to a cross-engine wait, and the MM ISA
    struct only has one sync-wait slot (walrus: "Too many sync wait
    commands"). A wait on sem S is redundant for instruction I on engine E iff
    S is only ever updated by E and the cumulative updates to S from E before
    I already reach the wait value.
    """
    import concourse.mybir as mybir

    for fn in nc.m.functions:
        for blk in fn.blocks:
            updaters = {}  # sem id -> set of engines updating it (block-wide)
            for i in blk.instructions:
                si = i.sync_info
                if si is None:
                    continue
                for u in si.on_update or []:
                    updaters.setdefault(u.id, set()).add(i.engine)
            seen = {}  # (engine, sem id) -> cumulative update count so far
            for i in blk.instructions:
                si = i.sync_info
                if si is None:
                    continue
                if si.on_wait:
                    kept = []
                    for w in si.on_wait:
                        if (
                            w.sync_type == "semaphore"
                            and w.wait_mode == "sem-ge-imm"
                            and updaters.get(w.id) == {i.engine}
                            and seen.get((i.engine, w.id), 0) >= w.wait_value
                        ):
                            continue  # implied by program order
                        kept.append(w)
                    if len(kept) != len(si.on_wait):
                        si.on_wait[:] = kept
                for u in si.on_update or []:
                    key = (i.engine, u.id)
                    seen[key] = seen.get(key, 0) + u.update_value
            # each engine ISA struct has a single sync-wait slot: offload
            # extra waits onto same-engine nops inserted just before
            new_insts = []
            nop_n = 0
            for i in blk.instructions:
                si = i.sync_info
                if si is not None and si.on_wait and len(si.on_wait) > 1:
                    extra = list(si.on_wait[:-1])
                    si.on_wait[:] = [si.on_wait[-1]]
                    for w in extra:
                        nop = mybir.InstNoOp(
                            name=f"I-waitnop-{blk.name}-{nop_n}", ins=[], outs=[]
                        )
                        nop_n += 1
                        nop.engine = i.engine
                        nop.sync_info = mybir.SyncInfo(on_wait=[w], on_update=[])
                        new_insts.append(nop)
                new_insts.append(i)
            if nop_n:
                blk.instructions[:] = new_insts


def _hoist_input_dmas(nc, n_hoist=34):
    """Move the first input DMAs ahead of the sync engine's entry barrier.

    The Tile/BSP prologue (entry EVSEM barrier + TENSOR_LOAD) delays the
    first dma_start by ~7us. The leading input DMAs have no waits (inputs
    are resident at NEFF start, dst tiles untouched), so issuing them first
    starts the HBM reads during the prologue.
    """
    import concourse.mybir as mybir

    blocks = nc.m.functions[0].blocks
    body = blocks[1]
    dmas = []
    for i in body.instructions:
        if type(i).__name__ == "InstDMACopy" and i.engine == mybir.EngineType.SP:
            si = i.sync_info
            if si is not None and si.on_wait:
                break  # stop at the first gated DMA
            dmas.append(i)
            if len(dmas) >= n_hoist:
                break
    if not dmas:
        return
    dset = set(id(x) for x in dmas)
    body.instructions[:] = [i for i in body.instructions if id(i) not in dset]
    # insert into the prologue block after the leading InstCall, ahead of
    # the entry barrier: the sync engine starts immediately, so these DMAs
    # issue at t~0 while the other engines are still loading their code
    pro = blocks[0].instructions
    pos = 1 if pro and type(pro[0]).__name__ == "InstCall" else 0
    pro[:] = pro[:pos] + dmas + pro[pos:]


def build_nc():
    import concourse.bass as bass
    import concourse.mybir as mybir
    from concourse.tile import TileContext

    bf16 = mybir.dt.bfloat16
    f32 = mybir.dt.float32

    nc = bass.Bass(enable_partition_id=False)
    vol_ext = nc.declare_dram_parameter("vol", [2, 96, 12288], bf16, isOutput=False)
    mzt_ext = nc.declare_dram_parameter("mzt", [96, 144], bf16, isOutput=False)
    mt_ext = nc.declare_dram_parameter("mt", [96, 192], bf16, isOutput=False)
    out_ext = nc.declare_dram_parameter("out", [128, 216, 192], bf16, isOutput=True)

    with TileContext(nc) as tc:
        with (
            tc.tile_pool(name="consts", bufs=1) as consts,
            tc.tile_pool(name="vols", bufs=2) as vols_pool,
            tc.tile_pool(name="l1", bufs=1) as l1_pool,
            tc.tile_pool(name="l2", bufs=2) as l2_pool,
            tc.tile_pool(name="stage", bufs=6) as stage_pool,
            tc.tile_pool(name="pab", bufs=2, space="PSUM") as pab_pool,
            tc.tile_pool(name="pc", bufs=2, space="PSUM") as pc_pool,
        ):
            mt = consts.tile([96, 192], bf16)
            nc.sync.dma_start(out=mt[:], in_=mt_ext[:])
            mzt = consts.tile([96, 144], bf16)
            nc.sync.dma_start(out=mzt[:], in_=mzt_ext[:])

            vols = []
            for p in range(2):
                vol = vols_pool.tile([96, 12288], bf16, name="vol")
                for ch in range(16):
                    nc.sync.dma_start(
                        out=vol[:, ch * 768 : (ch + 1) * 768],
                        in_=vol_ext[p, :, ch * 768 : (ch + 1) * 768],
                    )
                vols.append(vol)

            # weighted ACT:DVE = 5:4 (throughput 1.2 vs 0.96 elem/ns)
            evac_state = [0]

            def evac(dst, src):
                i = evac_state[0] % 9
                evac_state[0] += 1
                if i % 2 == 0:
                    nc.scalar.copy(dst, src)
                else:
                    nc.vector.tensor_copy(dst, src)

            # output DMA issue rotation (hwdge sequencers)
            dma_state = [0]
            dma_engines = None  # set below once nc exists

            def out_dma(dst, src):
                i = dma_state[0] % 2
                dma_state[0] += 1
                eng = (nc.sync, nc.scalar)[i]
                eng.dma_start(out=dst, in_=src)

            chunk_base = [0, 72]  # global chunk offset of each pass

            def make_emit(p, L2f):
                """Returns emit_tile(ti): 4-chunk C tile -> stage -> DMA."""
                # stage table: tile index -> (stage_first_tile, stage_ntiles,
                # stage_c0) ; stages are STAGE_PLAN[p] chunks each
                plan = STAGE_PLAN[p]
                tile2stage = {}
                c0 = 0
                for s, nch in enumerate(plan):
                    nt = nch // 4
                    t0 = c0 // 4
                    for k in range(nt):
                        tile2stage[t0 + k] = (t0, nt, c0, nch)
                    c0 += nch
                stage_tiles = {}

                def emit_tile(ti):
                    t0, nt, c0, nch = tile2stage[ti]
                    if ti == t0:
                        stage_tiles[t0] = stage_pool.tile(
                            [128, nch, 192], bf16, name="stage"
                        )
                    stage = stage_tiles[t0]
                    pc = pc_pool.tile(
                        [128, 2, 2, 192], f32, name="pc",
                        padded_shape=[128, 2, 2, 256],
                    )
                    for k in range(4):
                        ch = ti * 4 + k
                        nc.tensor.matmul(
                            pc[:, k // 2, k % 2, :],
                            lhsT=L2f[:, ch * 128 : (ch + 1) * 128],
                            rhs=mt[:],
                            start=True,
                            stop=True,
                        )
                    off = (ti - t0) * 4
                    evac(
                        stage[:, off : off + 4, :].rearrange(
                            "q (b j) y -> q b j y", b=2
                        ),
                        pc[:, :, :, :],
                    )
                    if ti == t0 + nt - 1:
                        gc0 = chunk_base[p] + c0
                        nsplit = 2 if nch >= 16 else 4
                        pstep = 128 // nsplit
                        for sp in range(nsplit):
                            r0, r1 = sp * pstep, (sp + 1) * pstep
                            out_dma(
                                out_ext[r0:r1, gc0 : gc0 + nch, :],
                                stage[r0:r1, :, :],
                            )

                return emit_tile

            carry = None  # (emit_tile, next_tile, total_tiles) from pass 0

            for p in range(2):
                w = PASS_W[p]
                G = 384 // w  # MMs per PSUM bank in stage A (8 or 4)
                moff = 0 if p == 0 else 48
                vol = vols[p]

                # ---- stage A: contract z -> L1[y, z', x] ----
                L1 = l1_pool.tile([96, w, 128], bf16, name="l1")
                nc.gpsimd.memset(L1[:, :, 96:128], 0.0)
                ngroups = 96 // (2 * G)
                for g in range(ngroups):
                    pa = pab_pool.tile(
                        [128, 2, w, G], f32, name="pa", tag="pab",
                        padded_shape=[128, 2, 512 // G, G],
                    )
                    for b in range(2):
                        for j in range(G):
                            x = g * 2 * G + b * G + j
                            nc.tensor.matmul(
                                pa[:, b, :, j],
                                lhsT=vol[:, x * 128 : (x + 1) * 128],
                                rhs=mzt[:, moff : moff + w],
                                start=True,
                                stop=True,
                            )
                    evac(
                        L1[:, :, g * 2 * G : (g + 1) * 2 * G].rearrange(
                            "q w (b g) -> q b w g", b=2
                        ),
                        pa[0:96, :, :, :],
                    )
                    # previous pass's held-back C tiles ride along with A
                    if carry is not None:
                        c_emit, c_next, c_tot = carry
                        c_emit(c_next)
                        carry = (c_emit, c_next + 1, c_tot) if c_next + 1 < c_tot else None
                while carry is not None:
                    c_emit, c_next, c_tot = carry
                    c_emit(c_next)
                    carry = (c_emit, c_next + 1, c_tot) if c_next + 1 < c_tot else None

                # ---- stages B and C, interleaved ----
                L2 = l2_pool.tile([96, w, 192], bf16, name="l2")
                L2f = L2[:].rearrange("q a b -> q (a b)")
                emit_tile = make_emit(p, L2f)
                tiles_total = PASS_CHUNKS[p] // 4
                hold = HOLD0 if p == 0 else 0
                t_next = 0
                for zz in range(w // 4):
                    pb = pab_pool.tile(
                        [128, 2, 2, 192], f32, name="pb", tag="pab",
                        padded_shape=[128, 2, 2, 256],
                    )
                    for b in range(2):
                        for jj in range(2):
                            zp = zz * 4 + b * 2 + jj
                            nc.tensor.matmul(
                                pb[:, b, jj, :],
                                lhsT=L1[:, zp, :],
                                rhs=mt[:],
                                start=True,
                                stop=True,
                            )
                    evac(
                        L2[:, zz * 4 : zz * 4 + 4, :].rearrange(
                            "q (b j) y -> q b j y", b=2
                        ),
                        pb[0:96, :, :, :],
                    )
                    rows_done = (zz + 1) * 4 * 192
                    while (
                        t_next < tiles_total - hold
                        and (t_next + 1) * 512 <= rows_done
                    ):
                        emit_tile(t_next)
                        t_next += 1
                while t_next < tiles_total - hold:
                    emit_tile(t_next)
                    t_next += 1
                carry = (emit_tile, t_next, tiles_total) if hold else None

    _strip_redundant_self_waits(nc)
    _hoist_input_dmas(nc)
    return nc


def _core_map(core):
    """Returns ((vol0, slab0), (vol1, half1)): pass0 48-row slab, pass1 96-row half."""
    return (4 + (core >= 4), core % 4), (core % 4, core // 4)


def make_in_maps(volume, M):
    mt_b = np.ascontiguousarray(M.T).astype(BF16)  # [96, 192]
    in_maps = []
    for core in range(NCORES):
        (v0, s0), (v1, h1) = _core_map(core)
        vols = np.zeros((2, 96, 96, 128), dtype=BF16)
        for slot, v in ((0, v0), (1, v1)):
            b, c = divmod(v, 3)
            vols[slot, :, :, :96] = np.transpose(volume[b, c], (0, 2, 1)).astype(BF16)
        vols = vols.reshape(2, 96, 12288)
        mzt = np.zeros((96, 144), dtype=BF16)
        mzt[:, 0:48] = M[s0 * 48 : (s0 + 1) * 48, :].T
        mzt[:, 48:144] = M[h1 * 96 : (h1 + 1) * 96, :].T
        in_maps.append({"vol": vols, "mzt": mzt, "mt": mt_b})
    return in_maps


def gather_out(results):
    out = np.zeros((2, 3, 192, 192, 192), dtype=np.float32)
    for core in range(NCORES):
        (v0, s0), (v1, h1) = _core_map(core)
        o = np.asarray(results[core]["out"], dtype=np.float32)  # [128, 216, 192]
        b, c = divmod(v0, 3)
        out[b, c, s0 * 48 : (s0 + 1) * 48] = (
            o[:, 0:72, :].transpose(1, 0, 2).reshape(48, 192, 192)
        )
        b, c = divmod(v1, 3)
        out[b, c, h1 * 96 : (h1 + 1) * 96] = (
            o[:, 72:216, :].transpose(1, 0, 2).reshape(96, 192, 192)
        )
    return out


def run(volume, trace=False):
    """Returns (output, exec_time_ns_or_None)."""
    import concourse.bass_utils as bu
    from concourse.bass_utils import run_bass_kernel_spmd

    if trace:
        # avoid the S3 artifact upload in the axon trace path
        bu.upload_artifacts = lambda tmpdir: str(tmpdir)

    volume = np.asarray(volume, dtype=np.float32)
    M = build_M()
    in_maps = make_in_maps(volume, M)
    if "nc" not in _NC_CACHE:
        _NC_CACHE["nc"] = build_nc()
    nc = _NC_CACHE["nc"]
    res = run_bass_kernel_spmd(
        nc, in_maps, core_ids=list(range(NCORES)), trace=trace
    )
    out = gather_out(res.results)
    return out, getattr(res, "exec_time_ns", None)


def kernel(volume):
    out, _ = run(volume, trace=False)
    return out


# revision 12
# speedup vs baseline: 1.1502x; 1.1502x over previous
"""Trainium2 Bass kernel: cubic B-spline upsampling x2 of a (2,3,96,96,96) volume.

Math: the reference op (recursive IIR prefilter along each spatial axis, then
an 8-tap stride-2 transposed conv along each axis) is linear and separable.
The whole per-axis operator is a dense 192x96 matrix M (built exactly on the
host in float64).  out = M (x) M (x) M applied along z, y, x.

Device strategy (8 NeuronCores, SPMD, no collectives): the 6 (b,c) volumes
x 192 z'-rows = 1152 output rows are split so every core gets 144 rows from
exactly TWO volumes, with a uniform program:
  pass 0: a 48-row z'-slab of volume 4 or 5   (slab  c%4,  vol 4+(c>=4))
  pass 1: a 96-row z'-half of volume 0..3     (half  c//4, vol c%4)
Each pass: load the volume once (96x96x96 bf16, unpadded), then three
data-stationary matmul stages (stationary = data tile, moving = spline
matrix), so no transposes and no padding anywhere (K=96 contractions):
  A: per x (96 mms): lhsT = vol[z, x-slice, y] (96x96) rhs=MzT(96,w) -> (y, z')
  B: per z' (w mms): lhsT = L1[y, z', x] (96x96)       rhs=MT (96,192) -> (x, y')
  C: per 128-chunk of (z'y') (1.5w mms): lhsT = L2f[x, chunk] rhs=MT -> (chunk, x')
PSUM->SBUF evacuations are 768-elem copies from 2-bank PSUM tiles, spread
ACT:DVE = 5:4 (their throughput ratio). Output DRAM layout is
partition-major [128, 216, 192] so every output DMA descriptor is a >=1.5KB
contiguous run (the 512B threshold below which DMA runs at half speed);
stage tiles of 16 chunks are DMA'd in 2-4 partition-splits, issued
round-robin from the sync/vector/scalar sequencers. Stage C of pass 0 is
partially held back to overlap pass 1's stage A; the last pass ends with
finer stages so the drain is short. Compute in bf16 (PSUM fp32); output
written bf16 and upcast on host (rel err ~5.3e-3 vs the reference).
"""

import math
import os
import sys

import numpy as np

for _p in ("/opt/trn_rl_repo",):
    if _p not in sys.path and os.path.isdir(_p):
        sys.path.insert(0, _p)

import ml_dtypes  # noqa: E402

BF16 = ml_dtypes.bfloat16

POLE = math.sqrt(3.0) - 2.0
GAIN = (1.0 - POLE) * (1.0 - 1.0 / POLE)  # 6.0
N = 96
F = 2
NOUT = N * F  # 192
NCORES = 8

PASS_W = (48, 96)  # z'-rows per pass
PASS_CHUNKS = (72, 144)  # w*192/128
# output stage sizes (chunks per staged DMA); finer at the very end
STAGE_PLAN = ((16, 16, 16, 16, 8), (16, 16, 16, 16, 16, 16, 16, 16, 8, 4, 4))
HOLD0 = 6  # 4-chunk C tiles of pass 0 held back into pass 1's stage A


def _cubic(t):
    a = np.abs(t)
    out = (2.0 / 3.0 + (0.5 * a - 1.0) * a**2) * (a < 1)
    out = out + (-((a - 2.0) ** 3) / 6.0) * ((a >= 1) & (a < 2))
    return out


def _prefilter_mat(n):
    """96x96 matrix of the causal+anticausal cubic-spline prefilter (float64)."""
    p = POLE
    xm = np.eye(n, dtype=np.float64) * GAIN
    i = np.arange(n)
    pows = p**i + p ** (2 * n - 1 - i)
    c = np.zeros((n, n), dtype=np.float64)
    c[0] = (pows @ xm) * (p / (1.0 - p ** (2 * n))) + xm[0]
    for k in range(1, n):
        c[k] = xm[k] + p * c[k - 1]
    out = np.zeros((n, n), dtype=np.float64)
    out[n - 1] = c[n - 1] * (p / (p - 1.0))
    for k in range(n - 2, -1, -1):
        out[k] = p * (out[k + 1] - c[k])
    return out


def _upsample_mat(n, f=F):
    """2n x n matrix of the edge-padded stride-2 transposed conv (float64)."""
    k = 4 * f  # f even -> is_odd == 0
    start = 1.0 / (2 * f) - 2.0
    pts = np.arange(k, dtype=np.float64) * (1.0 / f) + start
    ker = _cubic(pts)
    npad = n + 4
    U = np.zeros((f * n, npad), dtype=np.float64)
    for o in range(f * n):
        for i in range(npad):
            s = o + (k - 1) - f * i
            if 0 <= s < k:
                U[o, i] += ker[s]
    Uc = np.zeros((f * n, n), dtype=np.float64)
    for i in range(npad):
        j = min(max(i - 2, 0), n - 1)
        Uc[:, j] += U[:, i]
    return Uc


def build_M():
    """Exact 192x96 per-axis operator (float64)."""
    return _upsample_mat(N) @ _prefilter_mat(N)


_NC_CACHE = {}


def _strip_redundant_self_waits(nc):
    """Drop sem waits that are trivially satisfied by same-engine program order.

    Tile's per-proc wait emission is not transitively minimal: a PE matmul can
    end up waiting on the PE's own semaphore (already guaranteed by in-order
    engine execution) in addition to a cross-engine wait, and the MM ISA
    struct only has one sync-wait slot (walrus: "Too many sync wait
    commands"). A wait on sem S is redundant for instruction I on engine E iff
    S is only ever updated by E and the cumulative updates to S from E before
    I already reach the wait value.
    """
    import concourse.mybir as mybir

    for fn in nc.m.functions:
        for blk in fn.blocks:
            updaters = {}  # sem id -> set of engines updating it (block-wide)
            for i in blk.instructions:
                si = i.sync_info
                if si is None:
                    continue
                for u in si.on_update or []:
                    updaters.setdefault(u.id, set()).add(i.engine)
            seen = {}  # (engine, sem id) -> cumulative update count so far
            for i in blk.instructions:
                si = i.sync_info
                if si is None:
                    continue
                if si.on_wait:
                    kept = []
                    for w in si.on_wait:
                        if (
                            w.sync_type == "semaphore"
                            and w.wait_mode == "sem-ge-imm"
                            and updaters.get(w.id) == {i.engine}
                            and seen.get((i.engine, w.id), 0) >= w.wait_value
                        ):
                            continue  # implied by program order
                        kept.append(w)
                    if len(kept) != len(si.on_wait):
                        si.on_wait[:] = kept
                for u in si.on_update or []:
                    key = (i.engine, u.id)
                    seen[key] = seen.get(key, 0) + u.update_value
            # each engine ISA struct has a single sync-wait slot: offload
            # extra waits onto same-engine nops inserted just before
            new_insts = []
            nop_n = 0
            for i in blk.instructions:
                si = i.sync_info
                if si is not None and si.on_wait and len(si.on_wait) > 1:
                    extra = list(si.on_wait[:-1])
                    si.on_wait[:] = [si.on_wait[-1]]
                    for w in extra:
                        nop = mybir.InstNoOp(
                            name=f"I-waitnop-{blk.name}-{nop_n}", ins=[], outs=[]
                        )
                        nop_n += 1
                        nop.engine = i.engine
                        nop.sync_info = mybir.SyncInfo(on_wait=[w], on_update=[])
                        new_insts.append(nop)
                new_insts.append(i)
            if nop_n:
                blk.instructions[:] = new_insts


def _hoist_input_dmas(nc, n_hoist=34):
    """Move the first input DMAs ahead of the sync engine's entry barrier.

    The Tile/BSP prologue (entry EVSEM barrier + TENSOR_LOAD) delays the
    first dma_start by ~7us. The leading input DMAs have no waits (inputs
    are resident at NEFF start, dst tiles untouched), so issuing them first
    starts the HBM reads during the prologue.
    """
    import concourse.mybir as mybir

    blocks = nc.m.functions[0].blocks
    body = blocks[1]
    dmas = []
    for i in body.instructions:
        if type(i).__name__ == "InstDMACopy" and i.engine == mybir.EngineType.SP:
            si = i.sync_info
            if si is not None and si.on_wait:
                break  # stop at the first gated DMA
            dmas.append(i)
            if len(dmas) >= n_hoist:
                break
    if not dmas:
        return
    dset = set(id(x) for x in dmas)
    body.instructions[:] = [i for i in body.instructions if id(i) not in dset]
    # insert into the prologue block after the leading InstCall, ahead of
    # the entry barrier: the sync engine starts immediately, so these DMAs
    # issue at t~0 while the other engines are still loading their code
    pro = blocks[0].instructions
    pos = 1 if pro and type(pro[0]).__name__ == "InstCall" else 0
    pro[:] = pro[:pos] + dmas + pro[pos:]


def build_nc():
    import concourse.bass as bass
    import concourse.mybir as mybir
    from concourse.tile import TileContext

    bf16 = mybir.dt.bfloat16
    f32 = mybir.dt.float32

    nc = bass.Bass(enable_partition_id=False)
    vol_ext = nc.declare_dram_parameter("vol", [2, 128, 12288], bf16, isOutput=False)
    mzt_ext = nc.declare_dram_parameter("mzt", [128, 144], bf16, isOutput=False)
    mt_ext = nc.declare_dram_parameter("mt", [128, 192], bf16, isOutput=False)
    out_ext = nc.declare_dram_parameter("out", [128, 216, 192], bf16, isOutput=True)

    with TileContext(nc) as tc:
        with (
            tc.tile_pool(name="consts", bufs=1) as consts,
            tc.tile_pool(name="vols", bufs=2) as vols_pool,
            tc.tile_pool(name="l1", bufs=1) as l1_pool,
            tc.tile_pool(name="l2", bufs=2) as l2_pool,
            tc.tile_pool(name="stage", bufs=6) as stage_pool,
            tc.tile_pool(name="pab", bufs=2, space="PSUM") as pab_pool,
            tc.tile_pool(name="pc", bufs=2, space="PSUM") as pc_pool,
        ):
            mt = consts.tile([128, 192], bf16)
            nc.sync.dma_start(out=mt[:], in_=mt_ext[:])
            mzt = consts.tile([128, 144], bf16)
            nc.sync.dma_start(out=mzt[:], in_=mzt_ext[:])

            vols = []
            for p in range(2):
                vol = vols_pool.tile([128, 12288], bf16, name="vol")
                for ch in range(16):
                    nc.sync.dma_start(
                        out=vol[:, ch * 768 : (ch + 1) * 768],
                        in_=vol_ext[p, :, ch * 768 : (ch + 1) * 768],
                    )
                vols.append(vol)

            # weighted ACT:DVE = 5:4 (throughput 1.2 vs 0.96 elem/ns)
            evac_state = [0]

            def evac(dst, src):
                i = evac_state[0] % 9
                evac_state[0] += 1
                if i % 2 == 0:
                    nc.scalar.copy(dst, src)
                else:
                    nc.vector.tensor_copy(dst, src)

            # output DMA issue rotation (hwdge sequencers)
            dma_state = [0]
            dma_engines = None  # set below once nc exists

            def out_dma(dst, src):
                i = dma_state[0] % 2
                dma_state[0] += 1
                eng = (nc.sync, nc.scalar)[i]
                eng.dma_start(out=dst, in_=src)

            chunk_base = [0, 72]  # global chunk offset of each pass

            def make_emit(p, L2f):
                """Returns emit_tile(ti): 4-chunk C tile -> stage -> DMA."""
                # stage table: tile index -> (stage_first_tile, stage_ntiles,
                # stage_c0) ; stages are STAGE_PLAN[p] chunks each
                plan = STAGE_PLAN[p]
                tile2stage = {}
                c0 = 0
                for s, nch in enumerate(plan):
                    nt = nch // 4
                    t0 = c0 // 4
                    for k in range(nt):
                        tile2stage[t0 + k] = (t0, nt, c0, nch)
                    c0 += nch
                stage_tiles = {}

                def emit_tile(ti):
                    t0, nt, c0, nch = tile2stage[ti]
                    if ti == t0:
                        stage_tiles[t0] = stage_pool.tile(
                            [128, nch, 192], bf16, name="stage"
                        )
                    stage = stage_tiles[t0]
                    pc = pc_pool.tile(
                        [128, 2, 2, 192], f32, name="pc",
                        padded_shape=[128, 2, 2, 256],
                    )
                    for k in range(4):
                        ch = ti * 4 + k
                        nc.tensor.matmul(
                            pc[:, k // 2, k % 2, :],
                            lhsT=L2f[:, ch * 128 : (ch + 1) * 128],
                            rhs=mt[:],
                            start=True,
                            stop=True,
                        )
                    off = (ti - t0) * 4
                    evac(
                        stage[:, off : off + 4, :].rearrange(
                            "q (b j) y -> q b j y", b=2
                        ),
                        pc[:, :, :, :],
                    )
                    if ti == t0 + nt - 1:
                        gc0 = chunk_base[p] + c0
                        nsplit = 2 if nch >= 16 else 4
                        pstep = 128 // nsplit
                        for sp in range(nsplit):
                            r0, r1 = sp * pstep, (sp + 1) * pstep
                            out_dma(
                                out_ext[r0:r1, gc0 : gc0 + nch, :],
                                stage[r0:r1, :, :],
                            )

                return emit_tile

            carry = None  # (emit_tile, next_tile, total_tiles) from pass 0

            for p in range(2):
                w = PASS_W[p]
                G = 384 // w  # MMs per PSUM bank in stage A (8 or 4)
                moff = 0 if p == 0 else 48
                vol = vols[p]

                # ---- stage A: contract z -> L1[y, z', x] ----
                L1 = l1_pool.tile([128, w, 128], bf16, name="l1")
                nc.gpsimd.memset(L1[:, :, 96:128], 0.0)
                ngroups = 96 // (2 * G)
                for g in range(ngroups):
                    pa = pab_pool.tile(
                        [128, 2, w, G], f32, name="pa", tag="pab",
                        padded_shape=[128, 2, 512 // G, G],
                    )
                    for b in range(2):
                        for j in range(G):
                            x = g * 2 * G + b * G + j
                            nc.tensor.matmul(
                                pa[:, b, :, j],
                                lhsT=vol[:, x * 128 : (x + 1) * 128],
                                rhs=mzt[:, moff : moff + w],
                                start=True,
                                stop=True,
                            )
                    evac(
                        L1[:, :, g * 2 * G : (g + 1) * 2 * G].rearrange(
                            "q w (b g) -> q b w g", b=2
                        ),
                        pa[:, :, :, :],
                    )
                    # previous pass's held-back C tiles ride along with A
                    if carry is not None:
                        c_emit, c_next, c_tot = carry
                        c_emit(c_next)
                        carry = (c_emit, c_next + 1, c_tot) if c_next + 1 < c_tot else None
                while carry is not None:
                    c_emit, c_next, c_tot = carry
                    c_emit(c_next)
                    carry = (c_emit, c_next + 1, c_tot) if c_next + 1 < c_tot else None

                # ---- stages B and C, interleaved ----
                L2 = l2_pool.tile([128, w, 192], bf16, name="l2")
                L2f = L2[:].rearrange("q a b -> q (a b)")
                emit_tile = make_emit(p, L2f)
                tiles_total = PASS_CHUNKS[p] // 4
                hold = HOLD0 if p == 0 else 0
                t_next = 0
                for zz in range(w // 4):
                    pb = pab_pool.tile(
                        [128, 2, 2, 192], f32, name="pb", tag="pab",
                        padded_shape=[128, 2, 2, 256],
                    )
                    for b in range(2):
                        for jj in range(2):
                            zp = zz * 4 + b * 2 + jj
                            nc.tensor.matmul(
                                pb[:, b, jj, :],
                                lhsT=L1[:, zp, :],
                                rhs=mt[:],
                                start=True,
                                stop=True,
                            )
                    evac(
                        L2[:, zz * 4 : zz * 4 + 4, :].rearrange(
                            "q (b j) y -> q b j y", b=2
                        ),
                        pb[:, :, :, :],
                    )
                    rows_done = (zz + 1) * 4 * 192
                    while (
                        t_next < tiles_total - hold
                        and (t_next + 1) * 512 <= rows_done
                    ):
                        emit_tile(t_next)
                        t_next += 1
                while t_next < tiles_total - hold:
                    emit_tile(t_next)
                    t_next += 1
                carry = (emit_tile, t_next, tiles_total) if hold else None

    _strip_redundant_self_waits(nc)
    _hoist_input_dmas(nc)
    return nc


def _core_map(core):
    """Returns ((vol0, slab0), (vol1, half1)): pass0 48-row slab, pass1 96-row half."""
    return (4 + (core >= 4), core % 4), (core % 4, core // 4)


def make_in_maps(volume, M):
    mt_b = np.zeros((128, 192), dtype=BF16)
    mt_b[:96] = np.ascontiguousarray(M.T).astype(BF16)
    in_maps = []
    for core in range(NCORES):
        (v0, s0), (v1, h1) = _core_map(core)
        vols = np.zeros((2, 128, 96, 128), dtype=BF16)
        for slot, v in ((0, v0), (1, v1)):
            b, c = divmod(v, 3)
            vols[slot, :96, :, :96] = np.transpose(volume[b, c], (0, 2, 1)).astype(BF16)
        vols = vols.reshape(2, 128, 12288)
        mzt = np.zeros((128, 144), dtype=BF16)
        mzt[:96, 0:48] = M[s0 * 48 : (s0 + 1) * 48, :].T
        mzt[:96, 48:144] = M[h1 * 96 : (h1 + 1) * 96, :].T
        in_maps.append({"vol": vols, "mzt": mzt, "mt": mt_b})
    return in_maps


def gather_out(results):
    out = np.zeros((2, 3, 192, 192, 192), dtype=np.float32)
    for core in range(NCORES):
        (v0, s0), (v1, h1) = _core_map(core)
        o = np.asarray(results[core]["out"], dtype=np.float32)  # [128, 216, 192]
        b, c = divmod(v0, 3)
        out[b, c, s0 * 48 : (s0 + 1) * 48] = (
            o[:, 0:72, :].transpose(1, 0, 2).reshape(48, 192, 192)
        )
        b, c = divmod(v1, 3)
        out[b, c, h1 * 96 : (h1 + 1) * 96] = (
            o[:, 72:216, :].transpose(1, 0, 2).reshape(96, 192, 192)
        )
    return out


def run(volume, trace=False):
    """Returns (output, exec_time_ns_or_None)."""
    import concourse.bass_utils as bu
    from concourse.bass_utils import run_bass_kernel_spmd

    if trace:
        # avoid the S3 artifact upload in the axon trace path
        bu.upload_artifacts = lambda tmpdir: str(tmpdir)

    volume = np.asarray(volume, dtype=np.float32)
    M = build_M()
    in_maps = make_in_maps(volume, M)
    if "nc" not in _NC_CACHE:
        _NC_CACHE["nc"] = build_nc()
    nc = _NC_CACHE["nc"]
    res = run_bass_kernel_spmd(
        nc, in_maps, core_ids=list(range(NCORES)), trace=trace
    )
    out = gather_out(res.results)
    return out, getattr(res, "exec_time_ns", None)


def kernel(volume):
    out, _ = run(volume, trace=False)
    return out


# revision 13
# speedup vs baseline: 1.3951x; 1.2130x over previous
"""Trainium2 Bass kernel: cubic B-spline upsampling x2 of a (2,3,96,96,96) volume.

Math: the reference op (recursive IIR prefilter along each spatial axis, then
an 8-tap stride-2 transposed conv along each axis) is linear and separable.
The whole per-axis operator is a dense 192x96 matrix M (built exactly on the
host in float64).  out = M (x) M (x) M applied along z, y, x.

Device strategy (8 NeuronCores, SPMD, no collectives): the 6 (b,c) volumes
x 192 z'-rows = 1152 output rows are split so every core gets 144 rows from
exactly TWO volumes, with a uniform program:
  pass 0: a 48-row z'-slab of volume 4 or 5   (slab  c%4,  vol 4+(c>=4))
  pass 1: a 96-row z'-half of volume 0..3     (half  c//4, vol c%4)
Each pass: load the volume once (96x96x96 bf16, unpadded), then three
data-stationary matmul stages (stationary = data tile, moving = spline
matrix), so no transposes and no padding anywhere (K=96 contractions):
  A: per x (96 mms): lhsT = vol[z, x-slice, y] (96x96) rhs=MzT(96,w) -> (y, z')
  B: per z' (w mms): lhsT = L1[y, z', x] (96x96)       rhs=MT (96,192) -> (x, y')
  C: per 128-chunk of (z'y') (1.5w mms): lhsT = L2f[x, chunk] rhs=MT -> (chunk, x')
PSUM->SBUF evacuations are 768-elem copies from 2-bank PSUM tiles, spread
ACT:DVE = 5:4 (their throughput ratio). Output DRAM layout is
partition-major [128, 216, 192] so every output DMA descriptor is a >=1.5KB
contiguous run (the 512B threshold below which DMA runs at half speed);
stage tiles of 16 chunks are DMA'd in 2-4 partition-splits, issued
round-robin from the sync/vector/scalar sequencers. Stage C of pass 0 is
partially held back to overlap pass 1's stage A; the last pass ends with
finer stages so the drain is short. Compute in bf16 (PSUM fp32); output
written bf16 and upcast on host (rel err ~5.3e-3 vs the reference).
"""

import math
import os
import sys

import numpy as np

for _p in ("/opt/trn_rl_repo",):
    if _p not in sys.path and os.path.isdir(_p):
        sys.path.insert(0, _p)

import ml_dtypes  # noqa: E402

BF16 = ml_dtypes.bfloat16

POLE = math.sqrt(3.0) - 2.0
GAIN = (1.0 - POLE) * (1.0 - 1.0 / POLE)  # 6.0
N = 96
F = 2
NOUT = N * F  # 192
NCORES = 8

PASS_W = (48, 96)  # z'-rows per pass
PASS_CHUNKS = (72, 144)  # w*192/128
# output stage sizes (chunks per staged DMA); finer at the very end
STAGE_PLAN = ((16, 16, 16, 16, 8), (16, 16, 16, 16, 16, 16, 16, 16, 8, 8))
HOLD0 = 6  # 4-chunk C tiles of pass 0 held back into pass 1's stage A


def _cubic(t):
    a = np.abs(t)
    out = (2.0 / 3.0 + (0.5 * a - 1.0) * a**2) * (a < 1)
    out = out + (-((a - 2.0) ** 3) / 6.0) * ((a >= 1) & (a < 2))
    return out


def _prefilter_mat(n):
    """96x96 matrix of the causal+anticausal cubic-spline prefilter (float64)."""
    p = POLE
    xm = np.eye(n, dtype=np.float64) * GAIN
    i = np.arange(n)
    pows = p**i + p ** (2 * n - 1 - i)
    c = np.zeros((n, n), dtype=np.float64)
    c[0] = (pows @ xm) * (p / (1.0 - p ** (2 * n))) + xm[0]
    for k in range(1, n):
        c[k] = xm[k] + p * c[k - 1]
    out = np.zeros((n, n), dtype=np.float64)
    out[n - 1] = c[n - 1] * (p / (p - 1.0))
    for k in range(n - 2, -1, -1):
        out[k] = p * (out[k + 1] - c[k])
    return out


def _upsample_mat(n, f=F):
    """2n x n matrix of the edge-padded stride-2 transposed conv (float64)."""
    k = 4 * f  # f even -> is_odd == 0
    start = 1.0 / (2 * f) - 2.0
    pts = np.arange(k, dtype=np.float64) * (1.0 / f) + start
    ker = _cubic(pts)
    npad = n + 4
    U = np.zeros((f * n, npad), dtype=np.float64)
    for o in range(f * n):
        for i in range(npad):
            s = o + (k - 1) - f * i
            if 0 <= s < k:
                U[o, i] += ker[s]
    Uc = np.zeros((f * n, n), dtype=np.float64)
    for i in range(npad):
        j = min(max(i - 2, 0), n - 1)
        Uc[:, j] += U[:, i]
    return Uc


def build_M():
    """Exact 192x96 per-axis operator (float64)."""
    return _upsample_mat(N) @ _prefilter_mat(N)


_NC_CACHE = {}


def _strip_redundant_self_waits(nc):
    """Drop sem waits that are trivially satisfied by same-engine program order.

    Tile's per-proc wait emission is not transitively minimal: a PE matmul can
    end up waiting on the PE's own semaphore (already guaranteed by in-order
    engine execution) in addition to a cross-engine wait, and the MM ISA
    struct only has one sync-wait slot (walrus: "Too many sync wait
    commands"). A wait on sem S is redundant for instruction I on engine E iff
    S is only ever updated by E and the cumulative updates to S from E before
    I already reach the wait value.
    """
    import concourse.mybir as mybir

    for fn in nc.m.functions:
        for blk in fn.blocks:
            updaters = {}  # sem id -> set of engines updating it (block-wide)
            for i in blk.instructions:
                si = i.sync_info
                if si is None:
                    continue
                for u in si.on_update or []:
                    updaters.setdefault(u.id, set()).add(i.engine)
            seen = {}  # (engine, sem id) -> cumulative update count so far
            for i in blk.instructions:
                si = i.sync_info
                if si is None:
                    continue
                if si.on_wait:
                    kept = []
                    for w in si.on_wait:
                        if (
                            w.sync_type == "semaphore"
                            and w.wait_mode == "sem-ge-imm"
                            and updaters.get(w.id) == {i.engine}
                            and seen.get((i.engine, w.id), 0) >= w.wait_value
                        ):
                            continue  # implied by program order
                        kept.append(w)
                    if len(kept) != len(si.on_wait):
                        si.on_wait[:] = kept
                for u in si.on_update or []:
                    key = (i.engine, u.id)
                    seen[key] = seen.get(key, 0) + u.update_value
            # each engine ISA struct has a single sync-wait slot: offload
            # extra waits onto same-engine nops inserted just before
            new_insts = []
            nop_n = 0
            for i in blk.instructions:
                si = i.sync_info
                if si is not None and si.on_wait and len(si.on_wait) > 1:
                    extra = list(si.on_wait[:-1])
                    si.on_wait[:] = [si.on_wait[-1]]
                    for w in extra:
                        nop = mybir.InstNoOp(
                            name=f"I-waitnop-{blk.name}-{nop_n}", ins=[], outs=[]
                        )
                        nop_n += 1
                        nop.engine = i.engine
                        nop.sync_info = mybir.SyncInfo(on_wait=[w], on_update=[])
                        new_insts.append(nop)
                new_insts.append(i)
            if nop_n:
                blk.instructions[:] = new_insts


def _hoist_input_dmas(nc, n_hoist=34):
    """Move the first input DMAs ahead of the sync engine's entry barrier.

    The Tile/BSP prologue (entry EVSEM barrier + TENSOR_LOAD) delays the
    first dma_start by ~7us. The leading input DMAs have no waits (inputs
    are resident at NEFF start, dst tiles untouched), so issuing them first
    starts the HBM reads during the prologue.
    """
    import concourse.mybir as mybir

    blocks = nc.m.functions[0].blocks
    body = blocks[1]
    dmas = []
    for i in body.instructions:
        if type(i).__name__ == "InstDMACopy" and i.engine == mybir.EngineType.SP:
            si = i.sync_info
            if si is not None and si.on_wait:
                break  # stop at the first gated DMA
            dmas.append(i)
            if len(dmas) >= n_hoist:
                break
    if not dmas:
        return
    dset = set(id(x) for x in dmas)
    body.instructions[:] = [i for i in body.instructions if id(i) not in dset]
    # insert into the prologue block after the leading InstCall, ahead of
    # the entry barrier: the sync engine starts immediately, so these DMAs
    # issue at t~0 while the other engines are still loading their code
    pro = blocks[0].instructions
    pos = 1 if pro and type(pro[0]).__name__ == "InstCall" else 0
    pro[:] = pro[:pos] + dmas + pro[pos:]


def build_nc():
    import concourse.bass as bass
    import concourse.mybir as mybir
    from concourse.tile import TileContext

    bf16 = mybir.dt.bfloat16
    f32 = mybir.dt.float32

    nc = bass.Bass(enable_partition_id=False)
    vol_ext = nc.declare_dram_parameter("vol", [2, 128, 12288], bf16, isOutput=False)
    mzt_ext = nc.declare_dram_parameter("mzt", [128, 144], bf16, isOutput=False)
    mt_ext = nc.declare_dram_parameter("mt", [128, 192], bf16, isOutput=False)
    out_ext = nc.declare_dram_parameter("out", [128, 216, 192], bf16, isOutput=True)

    with TileContext(nc) as tc:
        with (
            tc.tile_pool(name="consts", bufs=1) as consts,
            tc.tile_pool(name="vols", bufs=2) as vols_pool,
            tc.tile_pool(name="l1", bufs=1) as l1_pool,
            tc.tile_pool(name="l2", bufs=2) as l2_pool,
            tc.tile_pool(name="stage", bufs=6) as stage_pool,
            tc.tile_pool(name="pab", bufs=2, space="PSUM") as pab_pool,
            tc.tile_pool(name="pc", bufs=2, space="PSUM") as pc_pool,
        ):
            mt = consts.tile([128, 192], bf16)
            nc.sync.dma_start(out=mt[:], in_=mt_ext[:])
            mzt = consts.tile([128, 144], bf16)
            nc.sync.dma_start(out=mzt[:], in_=mzt_ext[:])

            vols = []
            for p in range(2):
                vol = vols_pool.tile([128, 12288], bf16, name="vol")
                for ch in range(16):
                    nc.sync.dma_start(
                        out=vol[:, ch * 768 : (ch + 1) * 768],
                        in_=vol_ext[p, :, ch * 768 : (ch + 1) * 768],
                    )
                vols.append(vol)

            # weighted ACT:DVE = 5:4 (throughput 1.2 vs 0.96 elem/ns)
            evac_state = [0]

            def evac(dst, src):
                i = evac_state[0] % 9
                evac_state[0] += 1
                if i % 2 == 0:
                    nc.scalar.copy(dst, src)
                else:
                    nc.vector.tensor_copy(dst, src)

            # output DMA issue rotation (hwdge sequencers)
            dma_state = [0]
            dma_engines = None  # set below once nc exists

            def out_dma(dst, src):
                i = dma_state[0] % 2
                dma_state[0] += 1
                eng = (nc.sync, nc.scalar)[i]
                eng.dma_start(out=dst, in_=src)

            chunk_base = [0, 72]  # global chunk offset of each pass

            def make_emit(p, L2f):
                """Returns emit_tile(ti): 4-chunk C tile -> stage -> DMA."""
                # stage table: tile index -> (stage_first_tile, stage_ntiles,
                # stage_c0) ; stages are STAGE_PLAN[p] chunks each
                plan = STAGE_PLAN[p]
                tile2stage = {}
                c0 = 0
                for s, nch in enumerate(plan):
                    nt = nch // 4
                    t0 = c0 // 4
                    for k in range(nt):
                        tile2stage[t0 + k] = (t0, nt, c0, nch)
                    c0 += nch
                stage_tiles = {}

                def emit_tile(ti):
                    t0, nt, c0, nch = tile2stage[ti]
                    if ti == t0:
                        stage_tiles[t0] = stage_pool.tile(
                            [128, nch, 192], bf16, name="stage"
                        )
                    stage = stage_tiles[t0]
                    pc = pc_pool.tile(
                        [128, 2, 2, 192], f32, name="pc",
                        padded_shape=[128, 2, 2, 256],
                    )
                    for k in range(4):
                        ch = ti * 4 + k
                        nc.tensor.matmul(
                            pc[:, k // 2, k % 2, :],
                            lhsT=L2f[:, ch * 128 : (ch + 1) * 128],
                            rhs=mt[:],
                            start=True,
                            stop=True,
                        )
                    off = (ti - t0) * 4
                    evac(
                        stage[:, off : off + 4, :].rearrange(
                            "q (b j) y -> q b j y", b=2
                        ),
                        pc[:, :, :, :],
                    )
                    if ti == t0 + nt - 1:
                        gc0 = chunk_base[p] + c0
                        nsplit = 2 if nch >= 16 else 4
                        pstep = 128 // nsplit
                        for sp in range(nsplit):
                            r0, r1 = sp * pstep, (sp + 1) * pstep
                            out_dma(
                                out_ext[r0:r1, gc0 : gc0 + nch, :],
                                stage[r0:r1, :, :],
                            )

                return emit_tile

            carry = None  # (emit_tile, next_tile, total_tiles) from pass 0

            for p in range(2):
                w = PASS_W[p]
                G = 384 // w  # MMs per PSUM bank in stage A (8 or 4)
                moff = 0 if p == 0 else 48
                vol = vols[p]

                # ---- stage A: contract z -> L1[y, z', x] ----
                L1 = l1_pool.tile([128, w, 128], bf16, name="l1")
                nc.gpsimd.memset(L1[:, :, 96:128], 0.0)
                ngroups = 96 // (2 * G)
                for g in range(ngroups):
                    pa = pab_pool.tile(
                        [128, 2, w, G], f32, name="pa", tag="pab",
                        padded_shape=[128, 2, 512 // G, G],
                    )
                    for b in range(2):
                        for j in range(G):
                            x = g * 2 * G + b * G + j
                            nc.tensor.matmul(
                                pa[:, b, :, j],
                                lhsT=vol[:, x * 128 : (x + 1) * 128],
                                rhs=mzt[:, moff : moff + w],
                                start=True,
                                stop=True,
                            )
                    evac(
                        L1[:, :, g * 2 * G : (g + 1) * 2 * G].rearrange(
                            "q w (b g) -> q b w g", b=2
                        ),
                        pa[:, :, :, :],
                    )
                    # previous pass's held-back C tiles ride along with A
                    if carry is not None:
                        c_emit, c_next, c_tot = carry
                        c_emit(c_next)
                        carry = (c_emit, c_next + 1, c_tot) if c_next + 1 < c_tot else None
                while carry is not None:
                    c_emit, c_next, c_tot = carry
                    c_emit(c_next)
                    carry = (c_emit, c_next + 1, c_tot) if c_next + 1 < c_tot else None

                # ---- stages B and C, interleaved ----
                L2 = l2_pool.tile([128, w, 192], bf16, name="l2")
                L2f = L2[:].rearrange("q a b -> q (a b)")
                emit_tile = make_emit(p, L2f)
                tiles_total = PASS_CHUNKS[p] // 4
                hold = HOLD0 if p == 0 else 0
                t_next = 0
                for zz in range(w // 4):
                    pb = pab_pool.tile(
                        [128, 2, 2, 192], f32, name="pb", tag="pab",
                        padded_shape=[128, 2, 2, 256],
                    )
                    for b in range(2):
                        for jj in range(2):
                            zp = zz * 4 + b * 2 + jj
                            nc.tensor.matmul(
                                pb[:, b, jj, :],
                                lhsT=L1[:, zp, :],
                                rhs=mt[:],
                                start=True,
                                stop=True,
                            )
                    evac(
                        L2[:, zz * 4 : zz * 4 + 4, :].rearrange(
                            "q (b j) y -> q b j y", b=2
                        ),
                        pb[:, :, :, :],
                    )
                    rows_done = (zz + 1) * 4 * 192
                    while (
                        t_next < tiles_total - hold
                        and (t_next + 1) * 512 <= rows_done
                    ):
                        emit_tile(t_next)
                        t_next += 1
                while t_next < tiles_total - hold:
                    emit_tile(t_next)
                    t_next += 1
                carry = (emit_tile, t_next, tiles_total) if hold else None

    _strip_redundant_self_waits(nc)
    _hoist_input_dmas(nc)
    return nc


def _core_map(core):
    """Returns ((vol0, slab0), (vol1, half1)): pass0 48-row slab, pass1 96-row half."""
    return (4 + (core >= 4), core % 4), (core % 4, core // 4)


def make_in_maps(volume, M):
    mt_b = np.zeros((128, 192), dtype=BF16)
    mt_b[:96] = np.ascontiguousarray(M.T).astype(BF16)
    in_maps = []
    for core in range(NCORES):
        (v0, s0), (v1, h1) = _core_map(core)
        vols = np.zeros((2, 128, 96, 128), dtype=BF16)
        for slot, v in ((0, v0), (1, v1)):
            b, c = divmod(v, 3)
            vols[slot, :96, :, :96] = np.transpose(volume[b, c], (0, 2, 1)).astype(BF16)
        vols = vols.reshape(2, 128, 12288)
        mzt = np.zeros((128, 144), dtype=BF16)
        mzt[:96, 0:48] = M[s0 * 48 : (s0 + 1) * 48, :].T
        mzt[:96, 48:144] = M[h1 * 96 : (h1 + 1) * 96, :].T
        in_maps.append({"vol": vols, "mzt": mzt, "mt": mt_b})
    return in_maps


def gather_out(results):
    out = np.zeros((2, 3, 192, 192, 192), dtype=np.float32)
    for core in range(NCORES):
        (v0, s0), (v1, h1) = _core_map(core)
        o = np.asarray(results[core]["out"], dtype=np.float32)  # [128, 216, 192]
        b, c = divmod(v0, 3)
        out[b, c, s0 * 48 : (s0 + 1) * 48] = (
            o[:, 0:72, :].transpose(1, 0, 2).reshape(48, 192, 192)
        )
        b, c = divmod(v1, 3)
        out[b, c, h1 * 96 : (h1 + 1) * 96] = (
            o[:, 72:216, :].transpose(1, 0, 2).reshape(96, 192, 192)
        )
    return out


def run(volume, trace=False):
    """Returns (output, exec_time_ns_or_None)."""
    import concourse.bass_utils as bu
    from concourse.bass_utils import run_bass_kernel_spmd

    if trace:
        # avoid the S3 artifact upload in the axon trace path
        bu.upload_artifacts = lambda tmpdir: str(tmpdir)

    volume = np.asarray(volume, dtype=np.float32)
    M = build_M()
    in_maps = make_in_maps(volume, M)
    if "nc" not in _NC_CACHE:
        _NC_CACHE["nc"] = build_nc()
    nc = _NC_CACHE["nc"]
    res = run_bass_kernel_spmd(
        nc, in_maps, core_ids=list(range(NCORES)), trace=trace
    )
    out = gather_out(res.results)
    return out, getattr(res, "exec_time_ns", None)


def kernel(volume):
    out, _ = run(volume, trace=False)
    return out
